# revision 18
# baseline (speedup 1.0000x reference)
"""Block-diagonal linear (segment_reduce) Trainium2 kernel.

y[b, o] = sum_k x[b, o*16 + k] * weight[o, k]
x: (8192, 32768) f32, weight: (2048, 16) f32 -> y: (8192, 2048) f32

Sharding: data-parallel over batch across 8 NeuronCores (1024 rows each);
weight replicated (broadcast across partitions on-chip by the otherwise-idle
TensorE instead of re-reading it 128x from HBM). Per core the kernel streams
x in (128, 8192) half-tiles cast to fp16 by the SWDGE DMA, multiplies by the
broadcast weight on the DVE (2x packed mode), and reduces each 16-element
segment with a binary tree whose first two levels run on the DVE and last
two on the otherwise-idle GPSIMD engine — keeping DVE time per tile well
under the HBM-bandwidth shadow so the x-stream DMA never stalls on buffer
reuse. y is stored as fp16 (upcast on the host) to halve output HBM traffic.
"""

import numpy as np

import concourse.bass as bass
import concourse.mybir as mybir
from concourse.bass_utils import run_bass_kernel_spmd
from concourse.tile import TileContext

B = 8192
IN_F = 32768
OUT_F = 2048
BLK = 16
N_CORES = 8
B_LOC = B // N_CORES  # 1024

CCHUNK = 16384              # feature columns per tile
SEG = CCHUNK // BLK         # outputs per tile (1024)
N_CC = IN_F // CCHUNK       # 2
N_BT = B_LOC // 128         # 8
HALF = CCHUNK // 2          # 8192
HSEG = HALF // BLK          # 512

F32 = mybir.dt.float32
F32R = mybir.dt.float32r
F16 = mybir.dt.float16

_NC_CACHE = {}


def _build(legalize=True, **bass_kwargs):
    key = ("nc", legalize, tuple(sorted(bass_kwargs.items())))
    if key in _NC_CACHE:
        return _NC_CACHE[key]
    nc = bass.Bass(**bass_kwargs)
    x = nc.declare_dram_parameter("x", [B_LOC, IN_F], F32, isOutput=False)
    w = nc.declare_dram_parameter("weight", [OUT_F, BLK], F32R, isOutput=False)
    wb = nc.declare_dram_parameter("wb", [128, CCHUNK], F16, isOutput=False)
    onesr = nc.declare_dram_parameter("onesr", [1, 128], F32R, isOutput=False)
    y = nc.declare_dram_parameter("y", [B_LOC, OUT_F], F16, isOutput=True)

    wf = w[:].rearrange("o k -> (o k)")  # (32768,) flat, f = o*16 + k

    with TileContext(nc) as tc:
        with (
            tc.tile_pool(name="wpool", bufs=1) as wpool,
            tc.tile_pool(name="wrowp", bufs=2) as wrowp,
            tc.tile_pool(name="xpool", bufs=4) as xpool,
            tc.tile_pool(name="ypool", bufs=4) as ypool,
            tc.tile_pool(name="const", bufs=1) as constp,
            tc.tile_pool(name="psb", bufs=4, space="PSUM") as psb,
        ):
            # cc=0 weights come host-pre-tiled as a [128, CCHUNK] fp16 input:
            # one 4 MB DMA that lands during the startup ramp while the x
            # stream is still spinning up, so the first multiply starts at
            # ~19 us instead of waiting ~48 us for a PE broadcast chain.
            wbt = wpool.tile([128, CCHUNK], F16, name="wbt")
            nc.sync.dma_start(out=wbt[:], in_=wb[:])
            ones = constp.tile([1, 128], F32R)
            nc.sync.dma_start(out=ones[:], in_=onesr[:])
            # Pre-issue tile 0's loads so the SWDGE queue starts streaming x
            # at t=0, concurrent with the weight loads.
            xtile0 = xpool.tile([128, CCHUNK], F16, name="xt", tag="xt")
            for g in range(2):
                nc.gpsimd.dma_start(
                    out=xtile0[:, g * HALF : (g + 1) * HALF],
                    in_=x[0:128, g * HALF : (g + 1) * HALF],
                )
            # Broadcast the weight across all 128 partitions with the PE:
            # wtile[p, f] = wrow[0, f] via a K=1 ones-column fp32r matmul
            # (saves 16 MiB/core of HBM re-reads vs a DMA broadcast). Four
            # independent half-tiles so the first multiply only waits for
            # the first half of the first chunk's broadcast.
            wtiles = {
                (0, 0): wbt[:, 0:HALF],
                (0, 1): wbt[:, HALF:CCHUNK],
            }
            for cc in range(1, N_CC):
                for g in range(2):
                    wtile = wpool.tile([128, HALF], F16, name=f"wt{cc}{g}")
                    for h in range(8):
                        wrow = wrowp.tile([1, HALF // 8], F32R, name="wr", tag="wr")
                        off = cc * CCHUNK + g * HALF + h * (HALF // 8)
                        nc.sync.dma_start(out=wrow[:], in_=wf[off : off + HALF // 8])
                        for s in range(HALF // 8 // 512):
                            wps = psb.tile([128, 512], F32)
                            nc.tensor.matmul(
                                out=wps[:, :],
                                lhsT=ones[:, 0:128],
                                rhs=wrow[:, s * 512 : (s + 1) * 512],
                                skip_group_check=True,
                            )
                            col = h * (HALF // 8) + s * 512
                            nc.scalar.copy(out=wtile[:, col : col + 512], in_=wps[:])
                    wtiles[(cc, g)] = wtile[:]
            with nc.allow_low_precision(
                reason="fp16 16-elem segment sums are ~1e-3 rel err, well "
                "inside the 2e-2 tolerance"
            ):
                for cc in range(N_CC):
                    for bt in range(N_BT):
                        # SWDGE DMA casts x to fp16 on the way in, so the
                        # DVE ops run in 2x packed mode; two half-tile DMAs
                        # so the first multiply waits for half the data.
                        if cc == 0 and bt == 0:
                            xtile = xtile0
                        else:
                            xtile = xpool.tile(
                                [128, CCHUNK], F16, name="xt", tag="xt"
                            )
                        for g in range(2):
                            xh = xtile[:, g * HALF : (g + 1) * HALF]
                            if not (cc == 0 and bt == 0):
                                nc.gpsimd.dma_start(
                                    out=xh,
                                    in_=x[
                                        bt * 128 : (bt + 1) * 128,
                                        cc * CCHUNK + g * HALF : cc * CCHUNK
                                        + (g + 1) * HALF,
                                    ],
                                )
                            nc.vector.tensor_mul(
                                out=xh, in0=xh, in1=wtiles[(cc, g)]
                            )
                            # Segmented 16 -> 1 reduction as a binary tree
                            # that telescopes in place on the DVE (all ops
                            # fp16 unit-inner-stride -> 2x packed mode).
                            p3 = xh.rearrange("p (s k) -> p s k", k=16)
                            l1 = xtile[
                                :, g * HALF : g * HALF + HALF // 2
                            ].rearrange("p (s k) -> p s k", k=8)
                            nc.vector.tensor_add(
                                out=l1, in0=p3[:, :, 0:8], in1=p3[:, :, 8:16]
                            )
                            l2 = xtile[
                                :, g * HALF : g * HALF + HALF // 4
                            ].rearrange("p (s k) -> p s k", k=4)
                            nc.vector.tensor_add(
                                out=l2, in0=l1[:, :, 0:4], in1=l1[:, :, 4:8]
                            )
                            l3 = xtile[
                                :, g * HALF : g * HALF + HALF // 8
                            ].rearrange("p (s k) -> p s k", k=2)
                            nc.vector.tensor_add(
                                out=l3, in0=l2[:, :, 0:2], in1=l2[:, :, 2:4]
                            )
                            ytile = ypool.tile(
                                [128, HSEG], F16, name="yt", tag="yt"
                            )
                            nc.vector.tensor_add(
                                out=ytile[:], in0=l3[:, :, 0], in1=l3[:, :, 1]
                            )
                            # Per-half store: the tail drains sooner and the
                            # store never waits on the other half's tree.
                            ycol = cc * SEG + g * HSEG
                            nc.sync.dma_start(
                                out=y[
                                    bt * 128 : (bt + 1) * 128,
                                    ycol : ycol + HSEG,
                                ],
                                in_=ytile[:],
                            )
    if legalize:
        _legalize_waits(nc)
        _audit_waits(nc)
    _NC_CACHE[key] = nc
    return nc


_ES_COUNTER = [0]


def _legalize_waits(nc):
    """walrus (this CoreV3 pin) accepts one sync wait per instruction (two on
    EventSemaphore); Tile sometimes emits more. Two fixes, in order:
      1. drop same-engine self-waits (a serial engine already executes its
         own stream in order, so a wait on its own proc lane is redundant);
      2. hoist still-excess waits onto EventSemaphore instructions inserted
         right before the offender on the same engine queue.
    """
    for b in nc.m.functions[0].blocks:
        il = b.instructions
        idx = 0
        while idx < len(il):
            i = il[idx]
            si = i.sync_info
            cap = 2 if i.opcode == "EventSemaphore" else 1
            if si is None or len(si.on_wait) <= cap:
                idx += 1
                continue
            eng = str(i.engine).split(".")[-1]
            keeps = []
            for w in si.on_wait:
                rest = None
                if w.ant_name.startswith(f"{eng}_sequencer_"):
                    rest = w.ant_name[len(eng) + 11 :]
                elif w.ant_name.startswith(f"{eng}_"):
                    rest = w.ant_name[len(eng) + 1 :]
                if rest is not None and rest.isdigit():
                    continue  # self-wait: implied by program order
                keeps.append(w)
            hoist, tail = keeps[:-cap], keeps[-cap:]
            while hoist:
                chunk, hoist = hoist[:2], hoist[2:]
                _ES_COUNTER[0] += 1
                es = mybir.InstEventSemaphore(
                    name=f"legalize-es-{_ES_COUNTER[0]}", ins=[], outs=[]
                )
                es.engine = i.engine
                es.sync_info = mybir.SyncInfo(on_wait=chunk, on_update=[])
                il.insert(idx, es)
                idx += 1
            i.sync_info = mybir.SyncInfo(on_wait=tail, on_update=list(si.on_update))
            idx += 1


def _audit_waits(nc):
    """walrus (CoreV3) accepts at most one sync wait per instruction
    (two on EventSemaphore). Fail at build time instead of compile time."""
    bad = []
    for b in nc.m.functions[0].blocks:
        for i in b.instructions:
            si = i.sync_info
            if si is None:
                continue
            cap = 2 if i.opcode == "EventSemaphore" else 1
            if len(si.on_wait) > cap:
                bad.append((i.name, i.opcode, len(si.on_wait)))
    if bad:
        raise AssertionError(f"instructions with too many waits: {bad[:10]}")


def _in_maps(x, weight):
    x = np.ascontiguousarray(np.asarray(x, dtype=np.float32))
    weight = np.ascontiguousarray(np.asarray(weight, dtype=np.float32))
    ones = np.ones((1, 128), dtype=np.float32)
    wb = np.ascontiguousarray(
        np.tile(weight.reshape(-1)[:CCHUNK].astype(np.float16), (128, 1))
    )
    return [
        {
            "x": x[i * B_LOC : (i + 1) * B_LOC],
            "weight": weight,
            "wb": wb,
            "onesr": ones,
        }
        for i in range(N_CORES)
    ]


def run(x, weight, **spmd_kwargs):
    nc = _build()
    res = run_bass_kernel_spmd(
        nc, _in_maps(x, weight), core_ids=list(range(N_CORES)), **spmd_kwargs
    )
    out = np.concatenate([r["y"] for r in res.results], axis=0).astype(np.float32)
    return out, res


def kernel(x, weight):
    out, _ = run(x, weight)
    return out


# revision 27
# speedup vs baseline: 1.0313x; 1.0313x over previous
"""Block-diagonal linear (segment_reduce) Trainium2 kernel.

y[b, o] = sum_k x[b, o*16 + k] * weight[o, k]
x: (8192, 32768) f32, weight: (2048, 16) f32 -> y: (8192, 2048) f32

Sharding: data-parallel over batch across 8 NeuronCores (1024 rows each);
weight replicated (broadcast across partitions on-chip by the otherwise-idle
TensorE instead of re-reading it 128x from HBM). Per core the kernel streams
x in (128, 8192) half-tiles cast to fp16 by the SWDGE DMA, multiplies by the
broadcast weight on the DVE (2x packed mode), and reduces each 16-element
segment with a binary tree whose first two levels run on the DVE and last
two on the otherwise-idle GPSIMD engine — keeping DVE time per tile well
under the HBM-bandwidth shadow so the x-stream DMA never stalls on buffer
reuse. y is stored as fp16 (upcast on the host) to halve output HBM traffic.
"""

import numpy as np

import concourse.bass as bass
import concourse.mybir as mybir
from concourse.bass_utils import run_bass_kernel_spmd
from concourse.tile import TileContext

B = 8192
IN_F = 32768
OUT_F = 2048
BLK = 16
N_CORES = 8
B_LOC = B // N_CORES  # 1024

CCHUNK = 16384              # feature columns per tile
SEG = CCHUNK // BLK         # outputs per tile (1024)
N_CC = IN_F // CCHUNK       # 2
N_BT = B_LOC // 128         # 8
HALF = CCHUNK // 2          # 8192
HSEG = HALF // BLK          # 512

F32 = mybir.dt.float32
F32R = mybir.dt.float32r
F16 = mybir.dt.float16

_NC_CACHE = {}


def _build(legalize=True, **bass_kwargs):
    key = ("nc", legalize, tuple(sorted(bass_kwargs.items())))
    if key in _NC_CACHE:
        return _NC_CACHE[key]
    nc = bass.Bass(**bass_kwargs)
    x = nc.declare_dram_parameter("x", [B_LOC, IN_F], F32, isOutput=False)
    w = nc.declare_dram_parameter("weight", [OUT_F, BLK], F32R, isOutput=False)
    wb = nc.declare_dram_parameter("wb", [128, HALF], F16, isOutput=False)
    onesr = nc.declare_dram_parameter("onesr", [1, 128], F32R, isOutput=False)
    y = nc.declare_dram_parameter("y", [B_LOC, OUT_F], F16, isOutput=True)

    wf = w[:].rearrange("o k -> (o k)")  # (32768,) flat, f = o*16 + k

    with TileContext(nc) as tc:
        with (
            tc.tile_pool(name="wpool", bufs=1) as wpool,
            tc.tile_pool(name="wrowp", bufs=3) as wrowp,
            tc.tile_pool(name="xpool", bufs=4) as xpool,
            tc.tile_pool(name="ypool", bufs=3) as ypool,
            tc.tile_pool(name="const", bufs=1) as constp,
            tc.tile_pool(name="psb", bufs=4, space="PSUM") as psb,
        ):
            # The first half-chunk's weights come host-pre-tiled as a
            # [128, 8192] fp16 input: one 2 MB DMA that lands during the
            # startup ramp while the x stream is still spinning up, so the
            # first multiply starts at ~20 us instead of waiting ~48 us for
            # a PE broadcast chain. The other three half-chunks are PE-
            # broadcast on-chip (zero HBM cost) with plenty of slack.
            wbt = wpool.tile([128, HALF], F16, name="wbt")
            nc.sync.dma_start(out=wbt[:], in_=wb[:])
            ones = constp.tile([1, 128], F32R)
            nc.sync.dma_start(out=ones[:], in_=onesr[:])
            # Pre-issue tile 0's loads so the SWDGE queue starts streaming x
            # at t=0, concurrent with the weight loads.
            xtile0 = xpool.tile([128, CCHUNK], F16, name="xt", tag="xt")
            for g in range(2):
                nc.gpsimd.dma_start(
                    out=xtile0[:, g * HALF : (g + 1) * HALF],
                    in_=x[0:128, g * HALF : (g + 1) * HALF],
                )
            # Broadcast the weight across all 128 partitions with the PE:
            # wtile[p, f] = wrow[0, f] via a K=1 ones-column fp32r matmul
            # (saves 16 MiB/core of HBM re-reads vs a DMA broadcast). Four
            # independent half-tiles so the first multiply only waits for
            # the first half of the first chunk's broadcast.
            wtiles = {(0, 0): wbt}
            for cc, g in ((0, 1), (1, 0), (1, 1)):
                wtile = wpool.tile([128, HALF], F16, name=f"wt{cc}{g}")
                for h in range(8):
                    wrow = wrowp.tile([1, HALF // 8], F32R, name="wr", tag="wr")
                    off = cc * CCHUNK + g * HALF + h * (HALF // 8)
                    nc.sync.dma_start(out=wrow[:], in_=wf[off : off + HALF // 8])
                    for s in range(HALF // 8 // 512):
                        wps = psb.tile([128, 512], F32)
                        nc.tensor.matmul(
                            out=wps[:, :],
                            lhsT=ones[:, 0:128],
                            rhs=wrow[:, s * 512 : (s + 1) * 512],
                            skip_group_check=True,
                        )
                        col = h * (HALF // 8) + s * 512
                        nc.scalar.copy(out=wtile[:, col : col + 512], in_=wps[:])
                wtiles[(cc, g)] = wtile
            with nc.allow_low_precision(
                reason="fp16 16-elem segment sums are ~1e-3 rel err, well "
                "inside the 2e-2 tolerance"
            ):
                def emit_chunk(xtile, cc, bt, c0, width, wap, ytag, skip_dma):
                    """Load + multiply + tree-reduce + store for x columns
                    [cc*CCHUNK+c0, +width) of batch rows [bt*128, +128).
                    The tree telescopes in place starting at column c0."""
                    xseg = xtile[:, c0 : c0 + width]
                    if not skip_dma:
                        nc.gpsimd.dma_start(
                            out=xseg,
                            in_=x[
                                bt * 128 : (bt + 1) * 128,
                                cc * CCHUNK + c0 : cc * CCHUNK + c0 + width,
                            ],
                        )
                    nc.vector.tensor_mul(out=xseg, in0=xseg, in1=wap)
                    # Segmented 16 -> 1 reduction as a binary tree that
                    # telescopes in place on the DVE (all ops fp16 with
                    # unit inner stride -> 2x packed mode).
                    p3 = xseg.rearrange("p (s k) -> p s k", k=16)
                    l1 = xtile[:, c0 : c0 + width // 2].rearrange(
                        "p (s k) -> p s k", k=8
                    )
                    nc.vector.tensor_add(
                        out=l1, in0=p3[:, :, 0:8], in1=p3[:, :, 8:16]
                    )
                    l2 = xtile[:, c0 : c0 + width // 4].rearrange(
                        "p (s k) -> p s k", k=4
                    )
                    nc.vector.tensor_add(
                        out=l2, in0=l1[:, :, 0:4], in1=l1[:, :, 4:8]
                    )
                    l3 = xtile[:, c0 : c0 + width // 8].rearrange(
                        "p (s k) -> p s k", k=2
                    )
                    nc.vector.tensor_add(
                        out=l3, in0=l2[:, :, 0:2], in1=l2[:, :, 2:4]
                    )
                    nseg = width // BLK
                    ytile = ypool.tile([128, HSEG], F16, name="yt", tag="yt")
                    nc.vector.tensor_add(
                        out=ytile[:, 0:nseg], in0=l3[:, :, 0], in1=l3[:, :, 1]
                    )
                    # Fine-grained store: the tail drains sooner and the
                    # store never waits on another chunk's tree.
                    ycol = cc * SEG + c0 // BLK
                    nc.sync.dma_start(
                        out=y[bt * 128 : (bt + 1) * 128, ycol : ycol + nseg],
                        in_=ytile[:, 0:nseg],
                    )

                for cc in range(N_CC):
                    for bt in range(N_BT):
                        if cc == 0 and bt == 0:
                            xtile = xtile0
                        else:
                            xtile = xpool.tile(
                                [128, CCHUNK], F16, name="xt", tag="xt"
                            )
                        last = cc == N_CC - 1 and bt == N_BT - 1
                        if not last:
                            for g in range(2):
                                emit_chunk(
                                    xtile,
                                    cc,
                                    bt,
                                    g * HALF,
                                    HALF,
                                    wtiles[(cc, g)][:],
                                    "yt",
                                    skip_dma=(cc == 0 and bt == 0),
                                )
                        else:
                            # Final tile at quarter granularity: the drain
                            # after the last x read is one quarter-chain
                            # (~5 us) instead of a half-chain (~10 us).
                            QW = CCHUNK // 4
                            for q in range(4):
                                wap = wtiles[(cc, q // 2)][
                                    :, (q % 2) * QW : (q % 2 + 1) * QW
                                ]
                                emit_chunk(
                                    xtile, cc, bt, q * QW, QW, wap, "ytq",
                                    skip_dma=False,
                                )
    if legalize:
        _legalize_waits(nc)
        _audit_waits(nc)
    _NC_CACHE[key] = nc
    return nc


_ES_COUNTER = [0]


def _legalize_waits(nc):
    """walrus (this CoreV3 pin) accepts one sync wait per instruction (two on
    EventSemaphore); Tile sometimes emits more. Two fixes, in order:
      1. drop same-engine self-waits (a serial engine already executes its
         own stream in order, so a wait on its own proc lane is redundant);
      2. hoist still-excess waits onto EventSemaphore instructions inserted
         right before the offender on the same engine queue.
    """
    for b in nc.m.functions[0].blocks:
        il = b.instructions
        idx = 0
        while idx < len(il):
            i = il[idx]
            si = i.sync_info
            cap = 2 if i.opcode == "EventSemaphore" else 1
            if si is None or len(si.on_wait) <= cap:
                idx += 1
                continue
            eng = str(i.engine).split(".")[-1]
            keeps = []
            for w in si.on_wait:
                rest = None
                if w.ant_name.startswith(f"{eng}_sequencer_"):
                    rest = w.ant_name[len(eng) + 11 :]
                elif w.ant_name.startswith(f"{eng}_"):
                    rest = w.ant_name[len(eng) + 1 :]
                if rest is not None and rest.isdigit():
                    continue  # self-wait: implied by program order
                keeps.append(w)
            hoist, tail = keeps[:-cap], keeps[-cap:]
            while hoist:
                chunk, hoist = hoist[:2], hoist[2:]
                _ES_COUNTER[0] += 1
                es = mybir.InstEventSemaphore(
                    name=f"legalize-es-{_ES_COUNTER[0]}", ins=[], outs=[]
                )
                es.engine = i.engine
                es.sync_info = mybir.SyncInfo(on_wait=chunk, on_update=[])
                il.insert(idx, es)
                idx += 1
            i.sync_info = mybir.SyncInfo(on_wait=tail, on_update=list(si.on_update))
            idx += 1


def _audit_waits(nc):
    """walrus (CoreV3) accepts at most one sync wait per instruction
    (two on EventSemaphore). Fail at build time instead of compile time."""
    bad = []
    for b in nc.m.functions[0].blocks:
        for i in b.instructions:
            si = i.sync_info
            if si is None:
                continue
            cap = 2 if i.opcode == "EventSemaphore" else 1
            if len(si.on_wait) > cap:
                bad.append((i.name, i.opcode, len(si.on_wait)))
    if bad:
        raise AssertionError(f"instructions with too many waits: {bad[:10]}")


def _in_maps(x, weight):
    x = np.ascontiguousarray(np.asarray(x, dtype=np.float32))
    weight = np.ascontiguousarray(np.asarray(weight, dtype=np.float32))
    ones = np.ones((1, 128), dtype=np.float32)
    wb = np.ascontiguousarray(
        np.tile(weight.reshape(-1)[:HALF].astype(np.float16), (128, 1))
    )
    return [
        {
            "x": x[i * B_LOC : (i + 1) * B_LOC],
            "weight": weight,
            "wb": wb,
            "onesr": ones,
        }
        for i in range(N_CORES)
    ]


def run(x, weight, **spmd_kwargs):
    nc = _build()
    res = run_bass_kernel_spmd(
        nc, _in_maps(x, weight), core_ids=list(range(N_CORES)), **spmd_kwargs
    )
    out = np.concatenate([r["y"] for r in res.results], axis=0).astype(np.float32)
    return out, res


def kernel(x, weight):
    out, _ = run(x, weight)
    return out
